# revision 28
# baseline (speedup 1.0000x reference)
"""KAN layer on 8 Trainium2 NeuronCores (Bass/Tile) — fp8 DoubleRow hybrid.

Computes out = x @ base_weight.T + silu(x) @ spline_weight.sum(-1).T
for x:[8192,1024] f32 -> out:[8192,1024] f32.

Strategy (self-contained, hardcoded for these shapes):
  * Pure data-parallel over batch: core r computes out[1024r:1024(r+1), :].
  * Host prep: spline g-sum (it collapses algebraically), bf16/e4m3
    casts, and DMA-friendly layouts.
  * The base matmul (x @ Wb^T) runs in fp8e4 DoubleRow perf mode: both
    operands quantized to e4m3 (x scaled 1/8, Wb scaled 8 - the scales
    cancel inside each product), two k-tiles contracted per matmul ->
    4 DoubleRow matmuls instead of 8 bf16 ones per output tile.
  * The spline matmul (the dominant term by norm) stays bf16: silu(x)
    on the Scalar engine from bf16 x.  Both chains accumulate into the
    same f32 PSUM bank.  End-to-end rel err ~7e-3 (gate is 2e-2).
  * Weights are stationary; x moves with N=512 columns per matmul.
  * Scratch matmuls at t=0 warm the HAM clock gate during initial DMA.
"""
import sys

for _p in ("/opt/trn_rl_repo",):
    if _p not in sys.path:
        sys.path.insert(0, _p)

import ml_dtypes
import numpy as np

import concourse.bass as bass  # noqa: F401  (bass must import before mybir use)
import concourse.mybir as mybir
import concourse.tile as tile
from concourse import bacc
from concourse.bass_utils import run_bass_kernel_spmd

P = 128
IN_F = 1024
OUT_F = 1024
N_CORES = 8
B_LOC = 8192 // N_CORES      # 1024 batch rows per core
KI = IN_F // P               # 8 k-tiles per operand half
TP = KI // 2                 # 4 DoubleRow k-tile pairs (base half)
OT = OUT_F // P              # 8 out-feature tiles
MC = 512                     # moving free dim per matmul (= 1 PSUM bank f32)
NMC = B_LOC // MC            # 2 m-chunks
XS = 8.0                     # fp8 scale split: x/8, Wb*8 (cancels in product)

F32 = mybir.dt.float32
BF16 = mybir.dt.bfloat16
FP8 = mybir.dt.float8e4
AF = mybir.ActivationFunctionType
DR = mybir.MatmulPerfMode.DoubleRow

_compiled = None


def _build_kernel():
    nc = bacc.Bacc(None, target_bir_lowering=False, num_devices=N_CORES)
    # x and spline weights arrive in two 1MB chunks each, fp8 base
    # weights in one — few large DMA instructions sidestep the
    # per-instruction completion-receipt serialization on the HWDGE ring.
    xt = nc.dram_tensor("xt", [P, KI, B_LOC], BF16, kind="ExternalInput")
    wsp = nc.dram_tensor("wsp", [4, P, 2, KI, P], BF16,
                         kind="ExternalInput")
    wb8 = nc.dram_tensor("wb8", [P, OT, TP, 2, P], FP8,
                         kind="ExternalInput")
    out = nc.dram_tensor("out", [OT, NMC, P, MC], BF16,
                         kind="ExternalOutput")

    with tile.TileContext(nc) as tc:
        with (
            tc.tile_pool(name="const", bufs=1) as const,
            tc.tile_pool(name="pwarm", bufs=1, space="PSUM") as pwarm,
            tc.tile_pool(name="psum", bufs=7, space="PSUM") as psum,
            tc.tile_pool(name="opool", bufs=4) as opool,
        ):
            # ---- PE warmup: ~3.4us of fine-grained N=256 scratch
            # matmuls flips the HAM clock gate to 8/8 during the initial
            # DMA and drains right as the first silu tile lands ----
            scr = const.tile([P, MC], BF16, name="scr")
            nc.vector.memset(scr[:], 0.0)
            pscr = pwarm.tile([P, MC], F32, name="pscr")
            for _ in range(16):
                nc.tensor.matmul(pscr[:, 0:256], scr[:, 0:P], scr[:, 0:256],
                                 start=True, stop=True)

            # x in [2, 2, 4]-k-tile chunks: the first silu tile unblocks
            # right as the warmup queue drains
            xck = [2, 2, 4]
            xco = [0, 2, 4]
            xab = [const.tile([P, xck[c], B_LOC], BF16, name=f"xc{c}")
                   for c in range(3)]
            sab = [const.tile([P, xck[c], B_LOC], BF16, name=f"sc{c}")
                   for c in range(3)]
            wssb = [const.tile([P, 2, KI, P], BF16, name=f"ws{c}")
                    for c in range(4)]
            wbsb = const.tile([P, OT, TP, 2, P], FP8, name="wb")
            x8sb = [const.tile([P, 2, B_LOC], FP8, name=f"x8_{t}")
                    for t in range(TP)]

            # ---- DMAs in consumption order (fp8 weights needed last:
            # each group ends with its fp8 matmuls) ----
            nc.sync.dma_start(xab[0][:], xt[:, 0:2, :])
            nc.sync.dma_start(wssb[0][:], wsp[0])
            nc.sync.dma_start(xab[1][:], xt[:, 2:4, :])
            nc.sync.dma_start(wssb[1][:], wsp[1])
            nc.sync.dma_start(xab[2][:], xt[:, 4:8, :])
            nc.sync.dma_start(wssb[2][:], wsp[2])
            nc.sync.dma_start(wssb[3][:], wsp[3])
            nc.sync.dma_start(wbsb[:], wb8[:])

            # ---- fp8 x: cast + 1/8 scale on the (early-idle) Vector
            # engine, pair-packed for DoubleRow ----
            for t in range(TP):
                c = min(t, 2)
                k0 = 2 * t - xco[c]
                for i in range(2):
                    nc.vector.tensor_scalar_mul(
                        x8sb[t][:, i, :], xab[c][:, k0 + i, :], 1.0 / XS)

            # ---- silu(x) on the Scalar engine; first k-tile in halves
            # so the first matmul unblocks earlier ----
            def xchunk(k):
                c = 0 if k < 2 else (1 if k < 4 else 2)
                return c, k - xco[c]

            for k in range(KI):
                c, kk = xchunk(k)
                if k == 0:
                    for mc in range(NMC):
                        msl = slice(MC * mc, MC * (mc + 1))
                        nc.scalar.activation(sab[c][:, kk, msl],
                                             xab[c][:, kk, msl], AF.Silu)
                else:
                    nc.scalar.activation(sab[c][:, kk, :],
                                         xab[c][:, kk, :], AF.Silu)

            # ---- fused accumulation per [128 o, 512 m] tile:
            #      8 bf16 matmuls (spline, silu-paced) first, then
            #      4 fp8 DoubleRow matmuls (base) ----
            for o in range(OT):
                for mc in range(NMC):
                    pt = psum.tile([P, MC], F32, name="pt")
                    msl = slice(MC * mc, MC * (mc + 1))
                    for k in range(KI):
                        c, kk = xchunk(k)
                        nc.tensor.matmul(
                            pt[:], wssb[o // 2][:, o % 2, k, :],
                            sab[c][:, kk, msl],
                            start=(k == 0), stop=False,
                        )
                    for t in range(TP):
                        nc.tensor.matmul(
                            pt[:], wbsb[:, o, t], x8sb[t][:, :, msl],
                            start=False, stop=(t == TP - 1), perf_mode=DR,
                        )
                    ot = opool.tile([P, MC], BF16, name="ot")
                    if mc == 0:
                        nc.vector.tensor_copy(ot[:], pt[:])
                    else:
                        nc.scalar.activation(ot[:], pt[:], AF.Copy)
                    nc.sync.dma_start(out[o, mc], ot[:])
    nc.compile()
    return nc


def _get_compiled():
    global _compiled
    if _compiled is None:
        _compiled = _build_kernel()
    return _compiled


def _shard_inputs(x, base_weight, spline_weight):
    """Full inputs -> 8 per-core in_maps (casts + layout)."""
    x = np.asarray(x, dtype=np.float32)
    base_weight = np.asarray(base_weight, dtype=np.float32)
    spline_weight = np.asarray(spline_weight, dtype=np.float32)

    base_t = base_weight.T                         # [in, out]
    ws_t = spline_weight.sum(-1).T                 # [in, out]

    # base weights: e4m3, scaled by 8, DoubleRow pair-packed, one block:
    # [p, o, t, i, j] = 8*base_t[256 t + 128 i + p, 128 o + j]
    wb8_host = np.ascontiguousarray(
        (base_t * XS).reshape(TP, 2, P, OT, P).transpose(2, 3, 0, 1, 4)
    )
    wb8_host = np.clip(wb8_host, -240, 240).astype(ml_dtypes.float8_e4m3)
    # spline weights: bf16, four chunks of 2 o-tiles:
    # [c, p, o', k, j] = ws_t[128 k + p, (2c + o') 128 + j]
    wsp_host = np.ascontiguousarray(
        ws_t.reshape(KI, P, 4, 2, P).transpose(2, 1, 3, 0, 4)
    ).astype(ml_dtypes.bfloat16)

    in_maps = []
    for core in range(N_CORES):
        xr = x[B_LOC * core:B_LOC * (core + 1)]    # [1024 b, 1024 in]
        # [p, k, m] = xr.T[128 k + p, m]; sliced into k-chunks on device
        xt_host = np.ascontiguousarray(
            xr.T.reshape(KI, P, B_LOC).transpose(1, 0, 2)
        ).astype(ml_dtypes.bfloat16)
        in_maps.append({"xt": xt_host, "wb8": wb8_host, "wsp": wsp_host})
    return in_maps


def _gather_output(results):
    out = np.empty((8192, 1024), dtype=np.float32)
    for core in range(N_CORES):
        oc = results[core]["out"].astype(np.float32)   # [O, mc, p, m]
        out[B_LOC * core:B_LOC * (core + 1)] = (
            oc.transpose(1, 3, 0, 2).reshape(B_LOC, OUT_F)
        )
    return out


def run(trace=False, **inputs):
    """Run on the 8 NeuronCores; returns (out, BassKernelResults)."""
    nc = _get_compiled()
    in_maps = _shard_inputs(**inputs)
    res = run_bass_kernel_spmd(
        nc, in_maps, core_ids=list(range(N_CORES)), trace=trace)
    return _gather_output(res.results), res


def kernel(**inputs) -> np.ndarray:
    out, _ = run(trace=False, **inputs)
    return out


# revision 29
# speedup vs baseline: 1.0087x; 1.0087x over previous
"""KAN layer on 8 Trainium2 NeuronCores (Bass/Tile) — fp8 DoubleRow hybrid.

Computes out = x @ base_weight.T + silu(x) @ spline_weight.sum(-1).T
for x:[8192,1024] f32 -> out:[8192,1024] f32.

Strategy (self-contained, hardcoded for these shapes):
  * Pure data-parallel over batch: core r computes out[1024r:1024(r+1), :].
  * Host prep: spline g-sum (it collapses algebraically), bf16/e4m3
    casts, and DMA-friendly layouts.
  * The base matmul (x @ Wb^T) runs in fp8e4 DoubleRow perf mode: both
    operands quantized to e4m3 (x scaled 1/8, Wb scaled 8 - the scales
    cancel inside each product), two k-tiles contracted per matmul ->
    4 DoubleRow matmuls instead of 8 bf16 ones per output tile.
  * The spline matmul (the dominant term by norm) stays bf16: silu(x)
    on the Scalar engine from bf16 x.  Both chains accumulate into the
    same f32 PSUM bank, spline (silu-paced) first.  Rel err ~7e-3.
  * Weights are stationary; x moves with N=512 columns per matmul.
  * Scratch matmuls at t=0 warm the HAM clock gate during initial DMA.
"""
import sys

for _p in ("/opt/trn_rl_repo",):
    if _p not in sys.path:
        sys.path.insert(0, _p)

import ml_dtypes
import numpy as np

import concourse.bass as bass  # noqa: F401  (bass must import before mybir use)
import concourse.mybir as mybir
import concourse.tile as tile
from concourse import bacc
from concourse.bass_utils import run_bass_kernel_spmd

P = 128
IN_F = 1024
OUT_F = 1024
N_CORES = 8
B_LOC = 8192 // N_CORES      # 1024 batch rows per core
KI = IN_F // P               # 8 k-tiles per operand half
TP = KI // 2                 # 4 DoubleRow k-tile pairs (base half)
OT = OUT_F // P              # 8 out-feature tiles
MC = 512                     # moving free dim per matmul (= 1 PSUM bank f32)
NMC = B_LOC // MC            # 2 m-chunks
XS = 8.0                     # fp8 scale split: x/8, Wb*8 (cancels in product)

F32 = mybir.dt.float32
BF16 = mybir.dt.bfloat16
FP8 = mybir.dt.float8e4
AF = mybir.ActivationFunctionType
DR = mybir.MatmulPerfMode.DoubleRow

_compiled = None


def _build_kernel():
    nc = bacc.Bacc(None, target_bir_lowering=False, num_devices=N_CORES)
    xt = nc.dram_tensor("xt", [KI, P, B_LOC], BF16, kind="ExternalInput")
    wb8 = nc.dram_tensor("wb8", [OT, P, TP, 2, P], FP8, kind="ExternalInput")
    wsp = nc.dram_tensor("wsp", [OT, P, KI, P], BF16, kind="ExternalInput")
    out = nc.dram_tensor("out", [OT, NMC, P, MC], BF16, kind="ExternalOutput")

    with tile.TileContext(nc) as tc:
        with (
            tc.tile_pool(name="const", bufs=1) as const,
            tc.tile_pool(name="pwarm", bufs=1, space="PSUM") as pwarm,
            tc.tile_pool(name="psum", bufs=7, space="PSUM") as psum,
            tc.tile_pool(name="opool", bufs=4) as opool,
        ):
            # ---- PE warmup: keep HAM busy during the initial DMA ----
            scr = const.tile([P, MC], BF16, name="scr")
            nc.vector.memset(scr[:], 0.0)
            pscr = pwarm.tile([P, MC], F32, name="pscr")
            for _ in range(12):
                nc.tensor.matmul(pscr[:], scr[:, 0:P], scr[:],
                                 start=True, stop=True)

            wbsb = [const.tile([P, TP, 2, P], FP8, name=f"wb{o}")
                    for o in range(OT)]
            wssb = [const.tile([P, KI, P], BF16, name=f"ws{o}")
                    for o in range(OT)]
            x8sb = [const.tile([P, 2, B_LOC], FP8, name=f"x8_{t}")
                    for t in range(TP)]
            xsb = [const.tile([P, B_LOC], BF16, name=f"x{k}") for k in range(KI)]
            ssb = [const.tile([P, B_LOC], BF16, name=f"s{k}") for k in range(KI)]

            # ---- DMAs in consumption order: x feeds the silu chain that
            # paces the early matmuls; fp8 base weights are needed last ----
            order = [("x", 0), ("ws", 0), ("x", 1), ("x", 2), ("x", 3),
                     ("ws", 1), ("x", 4), ("x", 5), ("x", 6), ("x", 7),
                     ("ws", 2), ("wb", 0), ("wb", 1), ("ws", 3), ("wb", 2),
                     ("ws", 4), ("wb", 3), ("ws", 5), ("wb", 4), ("ws", 6),
                     ("wb", 5), ("ws", 7), ("wb", 6), ("wb", 7)]
            for kind, i in order:
                if kind == "x":
                    nc.sync.dma_start(xsb[i][:], xt[i])
                elif kind == "wb":
                    nc.sync.dma_start(wbsb[i][:], wb8[i])
                else:
                    nc.sync.dma_start(wssb[i][:], wsp[i])

            # ---- fp8 x: cast + 1/8 scale on the (early-idle) Vector
            # engine, pair-packed for DoubleRow ----
            for t in range(TP):
                for i in range(2):
                    nc.vector.tensor_scalar_mul(
                        x8sb[t][:, i, :], xsb[2 * t + i][:], 1.0 / XS)

            # ---- silu(x) on the Scalar engine.  The first two k-tiles
            # run at [128,512] so the first matmuls unblock ~0.4us
            # earlier; the rest run full-row for ACT throughput. ----
            for k in range(KI):
                if k < 2:
                    for mc in range(NMC):
                        msl = slice(MC * mc, MC * (mc + 1))
                        nc.scalar.activation(ssb[k][:, msl], xsb[k][:, msl],
                                             AF.Silu)
                else:
                    nc.scalar.activation(ssb[k][:], xsb[k][:], AF.Silu)

            # ---- fused accumulation per [128 o, 512 m] tile:
            #      8 bf16 matmuls (spline, silu-paced) first, then
            #      4 fp8 DoubleRow matmuls (base) ----
            for o in range(OT):
                for mc in range(NMC):
                    pt = psum.tile([P, MC], F32, name="pt")
                    msl = slice(MC * mc, MC * (mc + 1))
                    for k in range(KI):
                        nc.tensor.matmul(
                            pt[:], wssb[o][:, k], ssb[k][:, msl],
                            start=(k == 0), stop=False,
                        )
                    for t in range(TP):
                        nc.tensor.matmul(
                            pt[:], wbsb[o][:, t], x8sb[t][:, :, msl],
                            start=False, stop=(t == TP - 1), perf_mode=DR,
                        )
                    ot = opool.tile([P, MC], BF16, name="ot")
                    if mc == 0:
                        nc.vector.tensor_copy(ot[:], pt[:])
                    else:
                        nc.scalar.activation(ot[:], pt[:], AF.Copy)
                    nc.sync.dma_start(out[o, mc], ot[:])
    nc.compile()
    return nc


def _get_compiled():
    global _compiled
    if _compiled is None:
        _compiled = _build_kernel()
    return _compiled


def _shard_inputs(x, base_weight, spline_weight):
    """Full inputs -> 8 per-core in_maps (casts + layout)."""
    x = np.asarray(x, dtype=np.float32)
    base_weight = np.asarray(base_weight, dtype=np.float32)
    spline_weight = np.asarray(spline_weight, dtype=np.float32)

    base_t = base_weight.T                         # [in, out]
    ws_t = spline_weight.sum(-1).T                 # [in, out]

    # base weights: e4m3, scaled by 8, DoubleRow pair-packed
    # [O, p, tp, i, j] = 8*base_t[256 tp + 128 i + p, 128 O + j]
    wb8_host = np.ascontiguousarray(
        (base_t * XS).reshape(TP, 2, P, OT, P).transpose(3, 2, 0, 1, 4)
    )
    wb8_host = np.clip(wb8_host, -240, 240).astype(ml_dtypes.float8_e4m3)
    # spline weights: bf16, [O, p, k, j] = ws_t[128 k + p, 128 O + j]
    wsp_host = np.ascontiguousarray(
        ws_t.reshape(KI, P, OT, P).transpose(2, 1, 0, 3)
    ).astype(ml_dtypes.bfloat16)

    in_maps = []
    for core in range(N_CORES):
        xr = x[B_LOC * core:B_LOC * (core + 1)]    # [1024 b, 1024 in]
        xt_host = np.ascontiguousarray(
            xr.T.reshape(KI, P, B_LOC)).astype(ml_dtypes.bfloat16)
        in_maps.append({"xt": xt_host, "wb8": wb8_host, "wsp": wsp_host})
    return in_maps


def _gather_output(results):
    out = np.empty((8192, 1024), dtype=np.float32)
    for core in range(N_CORES):
        oc = results[core]["out"].astype(np.float32)   # [O, mc, p, m]
        out[B_LOC * core:B_LOC * (core + 1)] = (
            oc.transpose(1, 3, 0, 2).reshape(B_LOC, OUT_F)
        )
    return out


def run(trace=False, **inputs):
    """Run on the 8 NeuronCores; returns (out, BassKernelResults)."""
    nc = _get_compiled()
    in_maps = _shard_inputs(**inputs)
    res = run_bass_kernel_spmd(
        nc, in_maps, core_ids=list(range(N_CORES)), trace=trace)
    return _gather_output(res.results), res


def kernel(**inputs) -> np.ndarray:
    out, _ = run(trace=False, **inputs)
    return out
